# revision 1
# baseline (speedup 1.0000x reference)
"""DispersionLoss (InfoNCE_l2 variant) on 8 Trainium2 NeuronCores.

Computes  log( E_{i!=j}[ exp(-||z_i - z_j||^2 / tau) ] )  for z [8192, 512] fp32.

Strategy
--------
Let y = z * sqrt(2/tau), sqy_i = ||y_i||^2. Then
    exp(-||z_i-z_j||^2/tau) = exp(y_i.y_j) * exp(-sqy_i/2) * exp(-sqy_j/2)
(the relu clamp in the reference only matters on the diagonal, which we mask).

The 8192x8192 pair matrix is tiled into a 16x16 grid of 512x512 blocks.
Using symmetry, each unordered off-diagonal block pair is computed once:
core c owns block-rows {c, c+8} and computes blocks
    (c,   c+d) for d=0..8   and   (c+8, c+8+d mod 16) for d=0..7
which partitions { diag blocks } + { unordered pairs } exactly across 8 cores
(17 block-tiles per core). Off-diag block sums get host weight 2, diag blocks
weight 1 (their true diagonal is masked on-device via an identity-matmul that
adds -50 to the pre-exp argument).

SPMD trick: every core receives y^T with its columns *rotated* by 512*c, so
the schedule (which local column block pairs with which local lhs block) is
identical on every core; only the data differs. The lhsT tiles are slices of
the same rotated y^T already resident in SBUF (local blocks L0 and L8).

Engine split per 512x512 block-tile (a "quad" of 4 psum banks):
  - TensorE: 16 bf16 matmuls (K=128) accumulate G = y_i.y_j into a
    [128, 2048] psum tile (+1 identity-matmul per bank on diag tiles).
  - ScalarE: one pure-Exp activation over the whole [128, 2048] psum tile
    into a bf16 SBUF tile E.
  - VectorE: one 2x-mode multiply EW = E * A_colblock (A_j = exp(-sqy_j/2)
    broadcast across the 4 banks), then 4 row-sum reduces into a [128, 68]
    stats buffer.
  - Host: applies the a_i row factors (stats is per-row), the block
    weights, and log(sum / (N*(N-1))).

The y input is laid out [16, 128, 4*512] (column-block major, contraction
chunk along the free dim) so each 512KB column block is one dense DMA,
interleaved with the A_colblock pieces in rough consumption order. Warm-up
matmuls on memset data run while the DMAs stream so the PE's HAM clock gate
is already open (full clock) when the real matmuls start.
"""

import math

import numpy as np
import ml_dtypes

TAU = 100.0
N = 8192
DIM = 512
NCORES = 8
BLK = 512          # block size (rows/cols of a block-tile)
NBLK = 16          # number of 512-blocks along each axis
P = 128
KCH = 4            # contraction chunks of 128
NQ = 17            # block-tiles per core
DIAG_QUADS = (0, 9)
DIAG_NEG = -50.0   # added to pre-exp argument on the true diagonal
N_WARMUP_MM = 4

_cache = {}


def _build_nc():
    import concourse.bacc as bacc
    import concourse.mybir as mybir
    from concourse.tile import TileContext

    bf16 = mybir.dt.bfloat16
    f32 = mybir.dt.float32
    Exp = mybir.ActivationFunctionType.Exp
    mult = mybir.AluOpType.mult
    X = mybir.AxisListType.X
    XYZWC = mybir.AxisListType.XYZWC

    nc = bacc.Bacc(trn_type="TRN2")

    y = nc.dram_tensor("y", [NBLK, P, KCH * BLK], bf16, kind="ExternalInput")
    acol = nc.dram_tensor("acol", [4, P, 4 * BLK], bf16, kind="ExternalInput")
    ident = nc.dram_tensor("ident", [P, P], bf16, kind="ExternalInput")
    dpat = nc.dram_tensor("dpat", [P, 4 * BLK], bf16, kind="ExternalInput")
    stats = nc.dram_tensor("stats", [P, 4 * NQ], f32, kind="ExternalOutput")

    # block-tile schedule: (lhs block index {0: local L0, 1: local L8}, local
    # col block, is_diag). Identical on every core thanks to the rotation.
    quads = (
        [(0, 0, True)]
        + [(0, L, False) for L in range(1, 9)]
        + [(1, 8, True)]
        + [(1, L, False) for L in range(9, 16)]
    )

    with TileContext(nc) as tc:
        with (
            tc.tile_pool(name="persist", bufs=1) as pp,
            tc.tile_pool(name="equad", bufs=4) as ep,
            tc.tile_pool(name="psum", bufs=2, space="PSUM") as psp,
        ):
            rhs = [
                pp.tile([P, KCH * BLK], bf16, tag=f"rhs_{L}", name=f"rhs_{L}")
                if L > 0
                else None
                for L in range(NBLK)
            ]
            rhs0 = [
                pp.tile([P, BLK], bf16, tag=f"rhs0_{k}", name=f"rhs0_{k}")
                for k in range(KCH)
            ]

            def rhs_ap(k, L):
                if L == 0:
                    return rhs0[k][:, :]
                return rhs[L][:, k * BLK : (k + 1) * BLK]

            def lhs_ap(lhs_idx, k, rt_):
                if lhs_idx == 0:
                    return rhs0[k][:, rt_ * P : (rt_ + 1) * P]
                return rhs[8][:, k * BLK + rt_ * P : k * BLK + (rt_ + 1) * P]
            acol_t = [
                pp.tile([P, 4 * BLK], bf16, tag=f"acol_{i}", name=f"acol_{i}")
                for i in range(4)
            ]
            ident_t = pp.tile([P, P], bf16, tag="ident", name="ident_t")
            dpat_t = pp.tile([P, 4 * BLK], bf16, tag="dpat", name="dpat_t")
            stats_t = pp.tile([P, 4 * NQ], f32, tag="stats", name="stats_t")
            wsrc_t = pp.tile([P, BLK], bf16, tag="wsrc", name="wsrc_t")

            # PE warm-up on memset data (no DMA dependency): opens the HAM
            # clock gate while the first column blocks stream in.
            nc.gpsimd.memset(wsrc_t[:], 0.0)
            wps = psp.tile([P, 4 * BLK], f32, tag="ps", name="warm_ps")
            for i in range(N_WARMUP_MM):
                nc.tensor.matmul(
                    wps[:, :BLK], wsrc_t[:, :P], wsrc_t[:], start=True, stop=True
                )

            for k in range(KCH):
                nc.sync.dma_start(rhs0[k][:], y[0][:, k * BLK : (k + 1) * BLK])
            nc.sync.dma_start(ident_t[:], ident[:, :])
            nc.sync.dma_start(dpat_t[:], dpat[:, :])
            nc.sync.dma_start(rhs[1][:], y[1])
            nc.sync.dma_start(rhs[2][:], y[2])
            nc.sync.dma_start(acol_t[0][:], acol[0])
            nc.sync.dma_start(rhs[3][:], y[3])
            nc.sync.dma_start(rhs[4][:], y[4])
            nc.sync.dma_start(acol_t[1][:], acol[1])
            nc.sync.dma_start(rhs[5][:], y[5])
            nc.sync.dma_start(rhs[6][:], y[6])
            nc.sync.dma_start(acol_t[2][:], acol[2])
            nc.sync.dma_start(rhs[7][:], y[7])
            nc.sync.dma_start(acol_t[3][:], acol[3])
            for L in range(8, NBLK):
                nc.sync.dma_start(rhs[L][:], y[L])

            for q, (lhs_idx, L, is_diag) in enumerate(quads):
                ps = psp.tile([P, 4 * BLK], f32, tag="ps", name=f"ps_{q}")
                for rt_ in range(4):
                    seg = ps[:, rt_ * BLK : (rt_ + 1) * BLK]
                    for k in range(KCH):
                        nc.tensor.matmul(
                            seg,
                            lhs_ap(lhs_idx, k, rt_),
                            rhs_ap(k, L),
                            start=(k == 0),
                            stop=(k == KCH - 1) and not is_diag,
                        )
                # diag masks after all k-matmuls so the dpat/ident DMAs are
                # off the critical path at kernel start
                if is_diag:
                    for rt_ in range(4):
                        nc.tensor.matmul(
                            ps[:, rt_ * BLK : (rt_ + 1) * BLK],
                            ident_t[:],
                            dpat_t[:, rt_ * BLK : (rt_ + 1) * BLK],
                            start=False,
                            stop=True,
                        )
                e = ep.tile([P, 4 * BLK], bf16, tag="e", name=f"e_{q}")
                ew = ep.tile([P, 4 * BLK], bf16, tag="ew", name=f"ew_{q}")
                a_b = acol_t[L // 4][:, None, (L % 4) * BLK : (L % 4 + 1) * BLK]
                if q < NQ - 2:
                    # quad-wide exp + A_j multiply (fewer instructions)
                    nc.scalar.activation(e[:], ps[:], Exp)
                    nc.vector.tensor_tensor(
                        ew[:].rearrange("p (r b) -> p r b", r=4),
                        e[:].rearrange("p (r b) -> p r b", r=4),
                        a_b.to_broadcast((P, 4, BLK)),
                        mult,
                    )
                    # one 3D reduce: [128, 4, 512] -> per-bank sums [128, 4]
                    nc.vector.reduce_sum(
                        stats_t[:, 4 * q : 4 * q + 4],
                        ew[:].rearrange("p (r b) -> p r b", r=4),
                        axis=X,
                    )
                else:
                    # first/last quads: per-bank chains so the post-exp work
                    # starts as soon as each bank's matmuls finish instead of
                    # waiting for the whole quad
                    for rt_ in range(4):
                        sl = slice(rt_ * BLK, (rt_ + 1) * BLK)
                        nc.scalar.activation(e[:, sl], ps[:, sl], Exp)
                        nc.vector.tensor_tensor(
                            ew[:, sl], e[:, sl], a_b[:, 0, :], mult
                        )
                        nc.vector.reduce_sum(
                            stats_t[:, 4 * q + rt_ : 4 * q + rt_ + 1],
                            ew[:, sl],
                            axis=X,
                        )

            nc.sync.dma_start(stats[:, :], stats_t[:])

    nc.compile()
    return nc


def _host_inputs(z: np.ndarray):
    """Build the per-core input maps from the full z [8192, 512] fp32."""
    bf16 = ml_dtypes.bfloat16
    z64 = z.astype(np.float64)
    s = math.sqrt(2.0 / TAU)
    yT64 = (z64 * s).T  # [512, 8192]
    sqy64 = (2.0 / TAU) * np.sum(z64 * z64, axis=1)  # [8192]
    v64 = -0.5 * sqy64  # -sqy_j / 2

    ident = np.eye(P, dtype=np.float32).astype(bf16)
    dpat = np.zeros((P, 4 * BLK), dtype=np.float32)
    for rt_ in range(4):
        for p in range(P):
            dpat[p, rt_ * BLK + rt_ * P + p] = DIAG_NEG
    dpat = dpat.astype(bf16)

    in_maps = []
    amaps = []
    for c in range(NCORES):
        yr = np.roll(yT64, -BLK * c, axis=1).astype(np.float32).astype(bf16)
        # [512, 8192] -> [L=16, p=128, k=4, c=512] -> [16, 128, 2048]
        yl = np.ascontiguousarray(
            yr.reshape(KCH, P, NBLK, BLK).transpose(2, 1, 0, 3).reshape(
                NBLK, P, KCH * BLK
            )
        )

        vr = np.roll(v64, -BLK * c)
        acol = np.ascontiguousarray(
            np.broadcast_to(
                np.exp(vr).astype(np.float32).astype(bf16)[None, :], (P, N)
            ).reshape(P, 4, 4 * BLK).transpose(1, 0, 2)
        )

        # host-side row factors a_i = exp(-sqy_i/2)
        a_rows64 = np.empty((8, P), dtype=np.float64)
        for rt in range(8):
            base = BLK * (c + 8 * (rt // 4)) + (rt % 4) * P
            a_rows64[rt] = np.exp(v64[base : base + P])
        amap = np.empty((P, 4 * NQ), dtype=np.float64)
        for q in range(NQ):
            lhs_idx = 0 if q < 9 else 1
            for rt_ in range(4):
                amap[:, 4 * q + rt_] = a_rows64[4 * lhs_idx + rt_]
        amaps.append(amap)

        in_maps.append(
            {
                "y": yl,
                "acol": acol,
                "ident": ident,
                "dpat": dpat,
            }
        )
    return in_maps, amaps


def _reduce(results, amaps) -> np.ndarray:
    wq = np.array([1.0 if q in DIAG_QUADS else 2.0 for q in range(NQ)])
    total = 0.0
    for out_map, amap in zip(results, amaps):
        st = out_map["stats"].astype(np.float64)  # [P, 4*NQ]
        per_q = (st * amap).sum(axis=0).reshape(NQ, 4).sum(axis=1)
        total += (wq * per_q).sum()
    mean = total / (float(N) * float(N - 1))
    return np.array(math.log(mean), dtype=np.float32)


def run(z: np.ndarray, trace: bool = False, tmpdir=None):
    from concourse.bass_utils import run_bass_kernel_spmd

    if "nc" not in _cache:
        _cache["nc"] = _build_nc()
    nc = _cache["nc"]
    in_maps, amaps = _host_inputs(np.asarray(z, dtype=np.float32))
    res = run_bass_kernel_spmd(
        nc, in_maps, core_ids=list(range(NCORES)), trace=trace, tmpdir=tmpdir
    )
    return _reduce(res.results, amaps), res


def kernel(z: np.ndarray) -> np.ndarray:
    out, _ = run(z, trace=False)
    return out



# revision 6
# speedup vs baseline: 1.4989x; 1.4989x over previous
"""DispersionLoss (InfoNCE_l2 variant) on 8 Trainium2 NeuronCores.

Computes  log( E_{i!=j}[ exp(-||z_i - z_j||^2 / tau) ] )  for z [8192, 512] fp32.

Strategy (v2: fp8 DoubleRow + bias-folded columns + DVE tree reduce)
-------------------------------------------------------------------
Let y = z * sqrt(2/tau).  exp(-||z_i-z_j||^2/tau) = exp(y_i.y_j + v_i + v_j)
with v_i = -||y_i||^2/2 (the relu clamp only matters on the diagonal, which
is corrected on the host).

Quantization: ydata = e4m3(S*y[:, :510]) with S=192.  The centered column
bias delta_j = v_j - vbar is folded INTO the matmul contraction via the two
freed dims (510, 511): lhs rows carry 240.0, rhs rows carry a greedy 2-term
e4m3 decomposition of S^2*delta_j/240.  So PSUM = S^2*(G~_ij + delta~_j) with
no extra instructions; ScalarE computes exp(PSUM/S^2) directly.

The 8192x8192 pair matrix is tiled into 16x16 blocks of 512x512; each
unordered block pair computed once (same rotation-SPMD coverage as before:
core c owns row blocks {c, c+8} x col blocks {c..c+8} / {c+8..c+15}).
Quads are [128-row strip x 4 col blocks] so one exp + one reduce covers 2048
columns sharing a single host row factor A_i = exp(vbar + delta~_i).

Engine split per quad:
  - TensorE: 8 fp8 DoubleRow matmuls (K=256 each) accumulate into a
    [128, 2048] psum tile (2 LDWEIGHTS; stationary reused across banks).
  - ScalarE: one Exp activation (scale=1/S^2) -> bf16 E in SBUF.
  - VectorE: 2-level bf16 pairwise-add tree + one 512-wide reduce_sum
    -> stats column (full quads); one 3D reduce for the leftover block.
  - Host: row factors, weight-2 everywhere, then subtracts the exactly-known
    (fp8 inputs are host-generated) diagonal-block sums and log().
"""

import math

import numpy as np
import ml_dtypes

TAU = 100.0
N = 8192
DIM = 512
NCORES = 8
BLK = 512
NBLK = 16
P = 128
S = 192.0            # fp8 data scale
NDATA = 510          # data dims (510, 511 carry the folded column bias)
NFULL = 16           # full quads per core
NSTAT = 20           # stats columns: 16 full + 4 leftover strips
N_WARMUP_MM = 4

_cache = {}


def _build_nc():
    import concourse.bacc as bacc
    import concourse.mybir as mybir
    from concourse.tile import TileContext

    f8 = mybir.dt.float8e4
    bf16 = mybir.dt.bfloat16
    f32 = mybir.dt.float32
    Exp = mybir.ActivationFunctionType.Exp
    add = mybir.AluOpType.add
    X = mybir.AxisListType.X
    DR = mybir.MatmulPerfMode.DoubleRow

    nc = bacc.Bacc(trn_type="TRN2")

    yl = nc.dram_tensor("yl", [2, P, 4, BLK], f8, kind="ExternalInput")
    yr = nc.dram_tensor("yr", [NBLK, P, 4, BLK], f8, kind="ExternalInput")
    stats = nc.dram_tensor("stats", [P, NSTAT], f32, kind="ExternalOutput")

    # schedule: (kind, lhs_local, strip, col_base, stats_col)
    sched = (
        [("full", 0, s, 0, s) for s in range(4)]
        + [("full", 0, s, 4, 4 + s) for s in range(4)]
        + [("small",)]
        + [("full", 1, s, 8, 8 + s) for s in range(4)]
        + [("full", 1, s, 12, 12 + s) for s in range(4)]
    )

    with TileContext(nc) as tc:
        with (
            tc.tile_pool(name="persist", bufs=1) as pp,
            tc.tile_pool(name="equad", bufs=2) as ep,
            tc.tile_pool(name="psum", bufs=2, space="PSUM") as psp,
        ):
            ylt = [
                pp.tile([P, 4, BLK], f8, tag=f"yl_{r}", name=f"yl_{r}")
                for r in range(2)
            ]
            yrt = [
                pp.tile([P, 4, BLK], f8, tag=f"yr_{L}", name=f"yr_{L}")
                for L in range(NBLK)
            ]
            stats_t = pp.tile([P, NSTAT], f32, tag="stats", name="stats_t")
            wl = pp.tile([P, 2, P], f8, tag="wl", name="wl")
            wr = pp.tile([P, 2, BLK], f8, tag="wr", name="wr")
            wa_in = pp.tile([P, 8], bf16, tag="wa_in", name="wa_in")
            wa_out = pp.tile([P, 8], bf16, tag="wa_out", name="wa_out")

            # PE warm-up (HAM clock gate) + ACT exp-table preload, both on
            # memset data so they run while the first DMAs stream.
            nc.gpsimd.memset(wl[:], 0.0)
            nc.gpsimd.memset(wr[:], 0.0)
            nc.gpsimd.memset(wa_in[:], 0.0)
            wps = psp.tile([P, 4 * BLK], f32, tag="ps", name="warm_ps")
            for _ in range(N_WARMUP_MM):
                nc.tensor.matmul(
                    wps[:, :BLK], wl[:], wr[:], start=True, stop=True,
                    perf_mode=DR,
                )
            nc.scalar.activation(wa_out[:], wa_in[:], Exp)

            # DMAs in consumption order
            nc.sync.dma_start(ylt[0][:], yl[0])
            for L in range(4):
                nc.sync.dma_start(yrt[L][:], yr[L])
            for L in range(4, 9):
                nc.sync.dma_start(yrt[L][:], yr[L])
            nc.sync.dma_start(ylt[1][:], yl[1])
            for L in range(9, NBLK):
                nc.sync.dma_start(yrt[L][:], yr[L])

            inv_s2 = 1.0 / (S * S)

            for item in sched:
                ps = psp.tile([P, 4 * BLK], f32, tag="ps", name="ps")
                if item[0] == "full":
                    _, rb, strip, base, col = item
                    for kc in range(2):
                        lhs = ylt[rb][
                            :, 2 * kc : 2 * kc + 2, strip * P : (strip + 1) * P
                        ]
                        for b in range(4):
                            nc.tensor.matmul(
                                ps[:, b * BLK : (b + 1) * BLK],
                                lhs,
                                yrt[base + b][:, 2 * kc : 2 * kc + 2, :],
                                start=(kc == 0),
                                stop=(kc == 1),
                                perf_mode=DR,
                            )
                    e = ep.tile([P, 4 * BLK], bf16, tag="e", name=f"e_{col}")
                    nc.scalar.activation(e[:], ps[:], Exp, scale=inv_s2)
                    t1 = ep.tile([P, 2 * BLK], bf16, tag="t1", name=f"t1_{col}")
                    nc.vector.tensor_tensor(
                        t1[:], e[:, : 2 * BLK], e[:, 2 * BLK :], add
                    )
                    t2 = ep.tile([P, BLK], bf16, tag="t2", name=f"t2_{col}")
                    nc.vector.tensor_tensor(
                        t2[:], t1[:, :BLK], t1[:, BLK:], add
                    )
                    nc.vector.reduce_sum(
                        stats_t[:, col : col + 1], t2[:], axis=X
                    )
                else:
                    # leftover col block 8 vs the 4 strips of row block 0:
                    # bank b holds strip b, so the reduce is per-bank.
                    for kc in range(2):
                        for b in range(4):
                            nc.tensor.matmul(
                                ps[:, b * BLK : (b + 1) * BLK],
                                ylt[0][:, 2 * kc : 2 * kc + 2, b * P : (b + 1) * P],
                                yrt[8][:, 2 * kc : 2 * kc + 2, :],
                                start=(kc == 0),
                                stop=(kc == 1),
                                perf_mode=DR,
                            )
                    e = ep.tile([P, 4 * BLK], bf16, tag="e", name="e_small")
                    nc.scalar.activation(e[:], ps[:], Exp, scale=inv_s2)
                    nc.vector.reduce_sum(
                        stats_t[:, NFULL : NFULL + 4],
                        e[:].rearrange("p (r b) -> p r b", r=4),
                        axis=X,
                    )

            # strip index of each lhs row strip used by the full quads is
            # encoded in the schedule; the small quad uses strips 0..3 of
            # row block 0 (handled on the host identically).
            nc.sync.dma_start(stats[:, :], stats_t[:])

    nc.compile()
    return nc


def _quantize_e4m3(x: np.ndarray) -> np.ndarray:
    """float64 -> TRN e4m3 (bias 7, max +-240) -> float64 of the stored value."""
    q = np.clip(x, -240.0, 240.0).astype(ml_dtypes.float8_e4m3)
    return q.astype(np.float64), q


def _host_prep(z: np.ndarray):
    """Quantize/fold inputs; returns per-core input maps + reduction data."""
    f8 = ml_dtypes.float8_e4m3
    z64 = z.astype(np.float64)
    y = z64 * math.sqrt(2.0 / TAU)            # [N, DIM]
    v = -0.5 * np.sum(y * y, axis=1)          # true v_i (all 512 dims)
    vbar = float(v.mean())
    delta = v - vbar

    # data dims quantized at scale S
    qdata64, qdata8 = _quantize_e4m3(S * y[:, :NDATA])   # [N, 510]

    # two-term e4m3 decomposition of S^2*delta/240 for the folded bias
    T = (S * S) * delta / 240.0
    b1_64, b1_8 = _quantize_e4m3(T)
    b2_64, b2_8 = _quantize_e4m3(T - b1_64)
    dtil = 240.0 * (b1_64 + b2_64) / (S * S)  # folded delta~ (exact)
    arow = np.exp(vbar + dtil)                # host row factors A_i

    # Q matrix [DIM, N] in fp8: data rows + 2 bias rows; lhs variant has 240s
    Qr = np.zeros((DIM, N), dtype=f8)
    Qr[:NDATA] = qdata8.T
    Qr[NDATA] = b1_8
    Qr[NDATA + 1] = b2_8
    Ql = Qr.copy()
    Ql[NDATA] = f8(240.0)
    Ql[NDATA + 1] = f8(240.0)

    def block_tile(Q, b):
        # [DIM, BLK] -> [kc=4, p=128, BLK] -> [128, 4, BLK]; dim = 128*kc + p
        blk = Q[:, b * BLK : (b + 1) * BLK]
        return np.ascontiguousarray(
            blk.reshape(4, P, BLK).transpose(1, 0, 2)
        )

    tiles_r = [block_tile(Qr, b) for b in range(NBLK)]

    in_maps = []
    for c in range(NCORES):
        yr_in = np.stack([tiles_r[(c + L) % NBLK] for L in range(NBLK)])
        yl_in = np.stack(
            [block_tile(Ql, c), block_tile(Ql, (c + 8) % NBLK)]
        )
        in_maps.append({"yl": yl_in, "yr": yr_in})

    # host-side diagonal-block correction, in u-units
    # u_ij = exp(G~_ij + dtil_i + dtil_j + 2 vbar), G~ from quantized data dims
    corr = 0.0
    for b in range(NBLK):
        cols = slice(b * BLK, (b + 1) * BLK)
        qb = qdata64[cols, :]                      # [BLK, 510]
        G = (qb @ qb.T) / (S * S)
        ee = np.exp(G + dtil[cols][None, :] + dtil[cols][:, None] + 2.0 * vbar)
        corr += ee.sum() + np.trace(ee)
    return in_maps, arow, vbar, corr


def _reduce(results, arow, vbar, corr) -> np.ndarray:
    """Draw = sum over cores/quads of 2 * dot(stats_col, A_rows)."""
    draw = 0.0
    for c, out_map in enumerate(results):
        st = out_map["stats"].astype(np.float64)  # [P, NSTAT]
        prng = np.arange(P)
        for q in range(NFULL):
            rb_abs = c if q < 8 else c + 8
            strip = q % 4
            rows = BLK * rb_abs + P * strip + prng
            draw += 2.0 * float(st[:, q] @ arow[rows])
        for s_ in range(4):
            rows = BLK * c + P * s_ + prng
            draw += 2.0 * float(st[:, NFULL + s_] @ arow[rows])
    w = math.exp(vbar) * draw - corr
    mean = w / (float(N) * float(N - 1))
    return np.array(math.log(mean), dtype=np.float32)


def run(z: np.ndarray, trace: bool = False, tmpdir=None):
    from concourse.bass_utils import run_bass_kernel_spmd

    if "nc" not in _cache:
        _cache["nc"] = _build_nc()
    nc = _cache["nc"]
    in_maps, arow, vbar, corr = _host_prep(np.asarray(z, dtype=np.float32))
    res = run_bass_kernel_spmd(
        nc, in_maps, core_ids=list(range(NCORES)), trace=trace, tmpdir=tmpdir
    )
    return _reduce(res.results, arow, vbar, corr), res


def kernel(z: np.ndarray) -> np.ndarray:
    out, _ = run(z, trace=False)
    return out


# revision 10
# speedup vs baseline: 1.5994x; 1.0670x over previous
"""DispersionLoss (InfoNCE_l2 variant) on 8 Trainium2 NeuronCores.

Computes  log( E_{i!=j}[ exp(-||z_i - z_j||^2 / tau) ] )  for z [8192, 512] fp32.

Strategy (v2: fp8 DoubleRow + bias-folded columns + DVE tree reduce)
-------------------------------------------------------------------
Let y = z * sqrt(2/tau).  exp(-||z_i-z_j||^2/tau) = exp(y_i.y_j + v_i + v_j)
with v_i = -||y_i||^2/2 (the relu clamp only matters on the diagonal, which
is corrected on the host).

Quantization: ydata = e4m3(S*y[:, :510]) with S=192.  The centered column
bias delta_j = v_j - vbar is folded INTO the matmul contraction via the two
freed dims (510, 511): lhs rows carry 240.0, rhs rows carry a greedy 2-term
e4m3 decomposition of S^2*delta_j/240.  So PSUM = S^2*(G~_ij + delta~_j) with
no extra instructions; ScalarE computes exp(PSUM/S^2) directly.

The 8192x8192 pair matrix is tiled into 16x16 blocks of 512x512; each
unordered block pair computed once (same rotation-SPMD coverage as before:
core c owns row blocks {c, c+8} x col blocks {c..c+8} / {c+8..c+15}).
Quads are [128-row strip x 4 col blocks] so one exp + one reduce covers 2048
columns sharing a single host row factor A_i = exp(vbar + delta~_i).

Engine split per quad:
  - TensorE: 8 fp8 DoubleRow matmuls (K=256 each) accumulate into a
    [128, 2048] psum tile (2 LDWEIGHTS; stationary reused across banks).
  - ScalarE: one Exp activation (scale=1/S^2) -> bf16 E in SBUF.
  - VectorE: 2-level bf16 pairwise-add tree + one 512-wide reduce_sum
    -> stats column (full quads); one 3D reduce for the leftover block.
  - Host: row factors, weight-2 everywhere, then subtracts the exactly-known
    (fp8 inputs are host-generated) diagonal-block sums and log().
"""

import math

import numpy as np
import ml_dtypes

TAU = 100.0
N = 8192
DIM = 512
NCORES = 8
BLK = 512
NBLK = 16
P = 128
S = 192.0            # fp8 data scale
NDATA = 510          # data dims (510, 511 carry the folded column bias)
NFULL = 16           # full quads per core
NSTAT = 20           # stats columns: 16 full + 4 leftover strips
N_WARMUP_MM = 4

_cache = {}


def _build_nc():
    import concourse.bacc as bacc
    import concourse.mybir as mybir
    from concourse.tile import TileContext

    f8 = mybir.dt.float8e4
    bf16 = mybir.dt.bfloat16
    f32 = mybir.dt.float32
    Exp = mybir.ActivationFunctionType.Exp
    add = mybir.AluOpType.add
    X = mybir.AxisListType.X
    DR = mybir.MatmulPerfMode.DoubleRow

    nc = bacc.Bacc(trn_type="TRN2")

    yl = nc.dram_tensor("yl", [2, P, 4, BLK], f8, kind="ExternalInput")
    yr = nc.dram_tensor("yr", [NBLK, P, 4, BLK], f8, kind="ExternalInput")
    stats = nc.dram_tensor("stats", [P, NSTAT], f32, kind="ExternalOutput")

    # schedule: (kind, lhs_local, strip, col_base, stats_col)
    sched = (
        [("full", 0, s, 0, s) for s in range(4)]
        + [("full", 0, s, 4, 4 + s) for s in range(4)]
        + [("small",)]
        + [("full", 1, s, 8, 8 + s) for s in range(4)]
        + [("full", 1, s, 12, 12 + s) for s in range(4)]
    )

    with TileContext(nc) as tc:
        with (
            tc.tile_pool(name="persist", bufs=1) as pp,
            tc.tile_pool(name="equad", bufs=3) as ep,
            tc.tile_pool(name="psum", bufs=2, space="PSUM") as psp,
        ):
            ylt = [
                pp.tile([P, 4, BLK], f8, tag=f"yl_{r}", name=f"yl_{r}")
                for r in range(2)
            ]
            yrt = [
                pp.tile([P, 4, BLK], f8, tag=f"yr_{L}", name=f"yr_{L}")
                for L in range(NBLK)
            ]
            stats_t = pp.tile([P, NSTAT], f32, tag="stats", name="stats_t")
            wl = pp.tile([P, 2, P], f8, tag="wl", name="wl")
            wr = pp.tile([P, 2, BLK], f8, tag="wr", name="wr")
            wa_in = pp.tile([P, 8], bf16, tag="wa_in", name="wa_in")
            wa_out = pp.tile([P, 8], bf16, tag="wa_out", name="wa_out")

            # PE warm-up (HAM clock gate) + ACT exp-table preload on memset
            # data, issued while the first DMAs stream.  Memsets go on the
            # vector engine (idle at start, up earlier than gpsimd).
            nc.vector.memset(wl[:], 0.0)
            nc.vector.memset(wr[:], 0.0)
            nc.vector.memset(wa_in[:], 0.0)
            wps = psp.tile([P, 4 * BLK], f32, tag="ps", name="warm_ps")
            for _ in range(N_WARMUP_MM):
                nc.tensor.matmul(
                    wps[:, :BLK], wl[:], wr[:], start=True, stop=True,
                    perf_mode=DR,
                )
            nc.scalar.activation(wa_out[:], wa_in[:], Exp)

            # DMAs in consumption order.  The first quad needs yl[0] and
            # yr[0..3]; ship those as kc-halves split across both HWDGE
            # queues (sync + scalar) so the first matmuls start sooner.
            def half(t, h):
                return t[:, 2 * h : 2 * h + 2, :]

            nc.sync.dma_start(half(ylt[0], 0), half(yl[0], 0))
            nc.scalar.dma_start(half(yrt[0], 0), half(yr[0], 0))
            nc.sync.dma_start(half(yrt[1], 0), half(yr[1], 0))
            nc.scalar.dma_start(half(yrt[2], 0), half(yr[2], 0))
            nc.sync.dma_start(half(yrt[3], 0), half(yr[3], 0))
            nc.scalar.dma_start(half(ylt[0], 1), half(yl[0], 1))
            nc.sync.dma_start(half(yrt[0], 1), half(yr[0], 1))
            nc.scalar.dma_start(half(yrt[1], 1), half(yr[1], 1))
            nc.sync.dma_start(half(yrt[2], 1), half(yr[2], 1))
            nc.scalar.dma_start(half(yrt[3], 1), half(yr[3], 1))
            for L in range(4, 9):
                nc.sync.dma_start(yrt[L][:], yr[L])
            nc.sync.dma_start(ylt[1][:], yl[1])
            for L in range(9, NBLK):
                nc.sync.dma_start(yrt[L][:], yr[L])

            inv_s2 = 1.0 / (S * S)

            for item in sched:
                ps = psp.tile([P, 4 * BLK], f32, tag="ps", name="ps")
                if item[0] == "full":
                    _, rb, strip, base, col = item
                    for kc in range(2):
                        lhs = ylt[rb][
                            :, 2 * kc : 2 * kc + 2, strip * P : (strip + 1) * P
                        ]
                        for b in range(4):
                            nc.tensor.matmul(
                                ps[:, b * BLK : (b + 1) * BLK],
                                lhs,
                                yrt[base + b][:, 2 * kc : 2 * kc + 2, :],
                                start=(kc == 0),
                                stop=(kc == 1),
                                perf_mode=DR,
                            )
                    e = ep.tile([P, 4 * BLK], bf16, tag="e", name=f"e_{col}")
                    if col >= 14:
                        # last quads: fold the row-sum into the activation
                        # (accumulator read ~0.3us) so nothing trails the
                        # final ACT but the stats DMA.
                        nc.scalar.activation(
                            e[:], ps[:], Exp, scale=inv_s2,
                            accum_out=stats_t[:, col : col + 1],
                        )
                    else:
                        nc.scalar.activation(e[:], ps[:], Exp, scale=inv_s2)
                        t1 = ep.tile(
                            [P, 2 * BLK], bf16, tag="t1", name=f"t1_{col}"
                        )
                        nc.vector.tensor_tensor(
                            t1[:], e[:, : 2 * BLK], e[:, 2 * BLK :], add
                        )
                        t2 = ep.tile([P, BLK], bf16, tag="t2", name=f"t2_{col}")
                        nc.vector.tensor_tensor(
                            t2[:], t1[:, :BLK], t1[:, BLK:], add
                        )
                        nc.vector.reduce_sum(
                            stats_t[:, col : col + 1], t2[:], axis=X
                        )
                else:
                    # leftover col block 8 vs the 4 strips of row block 0:
                    # bank b holds strip b, so the reduce is per-bank.
                    for kc in range(2):
                        for b in range(4):
                            nc.tensor.matmul(
                                ps[:, b * BLK : (b + 1) * BLK],
                                ylt[0][:, 2 * kc : 2 * kc + 2, b * P : (b + 1) * P],
                                yrt[8][:, 2 * kc : 2 * kc + 2, :],
                                start=(kc == 0),
                                stop=(kc == 1),
                                perf_mode=DR,
                            )
                    e = ep.tile([P, 4 * BLK], bf16, tag="e", name="e_small")
                    nc.scalar.activation(e[:], ps[:], Exp, scale=inv_s2)
                    nc.vector.reduce_sum(
                        stats_t[:, NFULL : NFULL + 4],
                        e[:].rearrange("p (r b) -> p r b", r=4),
                        axis=X,
                    )

            # strip index of each lhs row strip used by the full quads is
            # encoded in the schedule; the small quad uses strips 0..3 of
            # row block 0 (handled on the host identically).
            nc.sync.dma_start(stats[:, :], stats_t[:])

    nc.compile()
    return nc


def _quantize_e4m3(x: np.ndarray) -> np.ndarray:
    """float64 -> TRN e4m3 (bias 7, max +-240) -> float64 of the stored value."""
    q = np.clip(x, -240.0, 240.0).astype(ml_dtypes.float8_e4m3)
    return q.astype(np.float64), q


def _host_prep(z: np.ndarray):
    """Quantize/fold inputs; returns per-core input maps + reduction data."""
    f8 = ml_dtypes.float8_e4m3
    z64 = z.astype(np.float64)
    y = z64 * math.sqrt(2.0 / TAU)            # [N, DIM]
    v = -0.5 * np.sum(y * y, axis=1)          # true v_i (all 512 dims)
    vbar = float(v.mean())
    delta = v - vbar

    # data dims quantized at scale S
    qdata64, qdata8 = _quantize_e4m3(S * y[:, :NDATA])   # [N, 510]

    # two-term e4m3 decomposition of S^2*delta/240 for the folded bias
    T = (S * S) * delta / 240.0
    b1_64, b1_8 = _quantize_e4m3(T)
    b2_64, b2_8 = _quantize_e4m3(T - b1_64)
    dtil = 240.0 * (b1_64 + b2_64) / (S * S)  # folded delta~ (exact)
    arow = np.exp(vbar + dtil)                # host row factors A_i

    # Q matrix [DIM, N] in fp8: data rows + 2 bias rows; lhs variant has 240s
    Qr = np.zeros((DIM, N), dtype=f8)
    Qr[:NDATA] = qdata8.T
    Qr[NDATA] = b1_8
    Qr[NDATA + 1] = b2_8
    Ql = Qr.copy()
    Ql[NDATA] = f8(240.0)
    Ql[NDATA + 1] = f8(240.0)

    def block_tile(Q, b):
        # [DIM, BLK] -> [kc=4, p=128, BLK] -> [128, 4, BLK]; dim = 128*kc + p
        blk = Q[:, b * BLK : (b + 1) * BLK]
        return np.ascontiguousarray(
            blk.reshape(4, P, BLK).transpose(1, 0, 2)
        )

    tiles_r = [block_tile(Qr, b) for b in range(NBLK)]

    in_maps = []
    for c in range(NCORES):
        yr_in = np.stack([tiles_r[(c + L) % NBLK] for L in range(NBLK)])
        yl_in = np.stack(
            [block_tile(Ql, c), block_tile(Ql, (c + 8) % NBLK)]
        )
        in_maps.append({"yl": yl_in, "yr": yr_in})

    # host-side diagonal-block correction, in u-units
    # u_ij = exp(G~_ij + dtil_i + dtil_j + 2 vbar), G~ from quantized data dims
    corr = 0.0
    for b in range(NBLK):
        cols = slice(b * BLK, (b + 1) * BLK)
        qb = qdata64[cols, :]                      # [BLK, 510]
        G = (qb @ qb.T) / (S * S)
        ee = np.exp(G + dtil[cols][None, :] + dtil[cols][:, None] + 2.0 * vbar)
        corr += ee.sum() + np.trace(ee)
    return in_maps, arow, vbar, corr


def _reduce(results, arow, vbar, corr) -> np.ndarray:
    """Draw = sum over cores/quads of 2 * dot(stats_col, A_rows)."""
    draw = 0.0
    for c, out_map in enumerate(results):
        st = out_map["stats"].astype(np.float64)  # [P, NSTAT]
        prng = np.arange(P)
        for q in range(NFULL):
            rb_abs = c if q < 8 else c + 8
            strip = q % 4
            rows = BLK * rb_abs + P * strip + prng
            draw += 2.0 * float(st[:, q] @ arow[rows])
        for s_ in range(4):
            rows = BLK * c + P * s_ + prng
            draw += 2.0 * float(st[:, NFULL + s_] @ arow[rows])
    w = math.exp(vbar) * draw - corr
    mean = w / (float(N) * float(N - 1))
    return np.array(math.log(mean), dtype=np.float32)


def run(z: np.ndarray, trace: bool = False, tmpdir=None):
    from concourse.bass_utils import run_bass_kernel_spmd

    if "nc" not in _cache:
        _cache["nc"] = _build_nc()
    nc = _cache["nc"]
    in_maps, arow, vbar, corr = _host_prep(np.asarray(z, dtype=np.float32))
    res = run_bass_kernel_spmd(
        nc, in_maps, core_ids=list(range(NCORES)), trace=trace, tmpdir=tmpdir
    )
    return _reduce(res.results, arow, vbar, corr), res


def kernel(z: np.ndarray) -> np.ndarray:
    out, _ = run(z, trace=False)
    return out
